# revision 1
# baseline (speedup 1.0000x reference)
"""Trainium2 Bass kernel for 16-head causal MHA (B=4, S=2048, D=1024).

Sharding: 8 cores = 4 batches x 2 head-groups (8 heads each).
Per core (batch b, head-group hg):
  inputs:  XT = X[b].T [1024,2048], WQ/WK/WV column shards [1024,512],
           WO row shard [512,1024], bias shards, causal mask tiles.
  output:  YT = (A_hg @ WO_hg + bo*[hg==0]).T  [1024, 2048]  (partial)
Host combine: Y[b] = (YT[2b] + YT[2b+1]).T

On-core dataflow (everything "transposed" so no on-device transposes):
  Q^T,K^T [512,2048] and V [2048,512] via fp32r matmuls;
  scores^T[sk,sq] = K_h @ Q_h^T; exp on ACT (scale=1/8 folded);
  causal mask multiplied on diagonal chunks;
  O^T_aug = [V_h | 1]^T @ attn^T  -> row 64 = softmax denominators;
  normalizer broadcast via K=1 matmul with ones; A^T scaled in place;
  Y^T = WO^T @ A^T (+bo as per-partition ACT bias).
"""

import sys

import numpy as np

_REPO = "/opt/trn_rl_repo"

B, S, D = 4, 2048, 1024
H, DK = 16, 64
HPC = 8            # heads per core
MD = HPC * DK      # 512: per-core head width
P = 128
SB = 512           # s-block
NSB = S // SB      # 4
NDC = D // P       # 8
NMC = MD // P      # 4
NSC = S // P       # 16
GRP = 2            # sk-chunks per exp group

_CACHE = {}


def _ensure_path():
    try:
        import concourse  # noqa: F401
    except ImportError:
        if _REPO not in sys.path:
            sys.path.insert(0, _REPO)


def _build():
    _ensure_path()
    from contextlib import ExitStack

    import concourse.bass as bass  # noqa: F401
    import concourse.mybir as mybir
    import concourse.tile as tile
    from concourse import bacc

    dt = mybir.dt
    f32 = dt.float32
    f32r = dt.float32r
    AF = mybir.ActivationFunctionType
    ALU = mybir.AluOpType

    nc = bacc.Bacc(None, target_bir_lowering=False)
    XT = nc.dram_tensor("XT", [D, S], f32r, kind="ExternalInput")
    WQ = nc.dram_tensor("WQ", [D, MD], f32r, kind="ExternalInput")
    WK = nc.dram_tensor("WK", [D, MD], f32r, kind="ExternalInput")
    WV = nc.dram_tensor("WV", [D, MD], f32r, kind="ExternalInput")
    WO = nc.dram_tensor("WO", [MD, D], f32r, kind="ExternalInput")
    BQ = nc.dram_tensor("BQ", [MD], f32, kind="ExternalInput")
    BK = nc.dram_tensor("BK", [MD], f32, kind="ExternalInput")
    BV = nc.dram_tensor("BV", [MD], f32r, kind="ExternalInput")
    BO = nc.dram_tensor("BO", [D], f32, kind="ExternalInput")
    MASKS = nc.dram_tensor("MASKS", [P, P], f32r, kind="ExternalInput")
    YT = nc.dram_tensor("YT", [D, S], f32, kind="ExternalOutput")

    with ExitStack() as ctx:
        ctx.enter_context(nc.allow_low_precision(reason="fp32r matmul pipeline"))
        tc = ctx.enter_context(tile.TileContext(nc))
        consts = ctx.enter_context(tc.tile_pool(name="consts", bufs=1))
        qkv = ctx.enter_context(tc.tile_pool(name="qkv", bufs=1))
        atp = ctx.enter_context(tc.tile_pool(name="atp", bufs=1))

        # Dummy first ACT op: walrus attaches the ACT table-load pseudo to the
        # first activation; keep its sync-wait list minimal.
        dummy = consts.tile([1, 16], f32)
        nc.vector.memset(dummy[:], 0.0)
        nc.scalar.activation(dummy[:], dummy[:], AF.Exp)
        nc.scalar.activation(dummy[:], dummy[:], AF.Identity)

        ones_t = consts.tile([1, P], f32r)
        nc.vector.memset(ones_t[:].bitcast(f32), 1.0)

        kt = qkv.tile([P, NMC, S], f32r)            # K^T  (m-chunk, sk)
        vaug = qkv.tile([P, NSC, HPC, DK + 1], f32r)  # V per s-chunk + ones col
        at = atp.tile([P, NMC, S], f32r)            # A^T accumulates heads

        with (
            tc.tile_pool(name="xt", bufs=1) as xtp,
            tc.tile_pool(name="wst", bufs=1) as wst,
            tc.tile_pool(name="qt", bufs=2) as qtp,
            tc.tile_pool(name="attn", bufs=3) as attnp,
            tc.tile_pool(name="rs", bufs=2) as rsp,
            tc.tile_pool(name="pp", bufs=2, space="PSUM") as pp,
            tc.tile_pool(name="psc", bufs=2, space="PSUM") as psc,
            tc.tile_pool(name="po", bufs=2, space="PSUM") as pop,
        ):
            for sb in range(NSB):
                # ---------- projections for s-block sb ----------
                xt_t = xtp.tile([P, NDC, SB], f32r)
                for dc in range(NDC):
                    nc.sync.dma_start(
                        xt_t[:, dc, :], XT[dc * P:(dc + 1) * P, sb * SB:(sb + 1) * SB]
                    )
                qt_t = qtp.tile([P, NMC, SB], f32r)

                if sb == 0:
                    wq_t = wst.tile([P, NDC, MD], f32r, tag="wq")
                    wq_r = WQ.rearrange("(c p) m -> p c m", p=P)
                    nc.sync.dma_start(wq_t[:, 0:1, :], wq_r[:, 0:1, :])
                    nc.sync.dma_start(wq_t[:, 1:3, :], wq_r[:, 1:3, :])
                    nc.sync.dma_start(wq_t[:, 3:NDC, :], wq_r[:, 3:NDC, :])
                    bqt = consts.tile([P, NMC], f32)
                    nc.sync.dma_start(bqt[:], BQ.rearrange("(c p) -> p c", p=P))
                    bkt = consts.tile([P, NMC], f32)
                    nc.sync.dma_start(bkt[:], BK.rearrange("(c p) -> p c", p=P))
                    bvt = consts.tile([1, MD], f32r)
                    nc.sync.dma_start(bvt[:], BV[None, :])
                    masks_t = consts.tile([P, P], f32r)
                    nc.sync.dma_start(masks_t[:], MASKS[:, :])
                    bot = consts.tile([P, NDC], f32)
                    nc.sync.dma_start(bot[:], BO.rearrange("(c p) -> p c", p=P))
                for mc in range(NMC):
                    ps = pp.tile([P, SB], f32)
                    for dc in range(NDC):
                        nc.tensor.matmul(
                            ps[:],
                            (wq_t[:, dc, mc * P:(mc + 1) * P]),
                            (xt_t[:, dc, :]),
                            start=(dc == 0),
                            stop=(dc == NDC - 1),
                        )
                    nc.vector.tensor_scalar_add(
                        qt_t[:, mc, :], ps[:], bqt[:, mc:mc + 1]
                    )

                if sb == 0:
                    wk_t = wst.tile([P, NDC, MD], f32r, tag="wk")
                    nc.sync.dma_start(wk_t[:], WK.rearrange("(c p) m -> p c m", p=P))
                for mc in range(NMC):
                    ps = pp.tile([P, SB], f32)
                    for dc in range(NDC):
                        nc.tensor.matmul(
                            ps[:],
                            (wk_t[:, dc, mc * P:(mc + 1) * P]),
                            (xt_t[:, dc, :]),
                            start=(dc == 0),
                            stop=(dc == NDC - 1),
                        )
                    nc.vector.tensor_scalar_add(
                        kt[:, mc, sb * SB:(sb + 1) * SB], ps[:], bkt[:, mc:mc + 1]
                    )

                if sb == 0:
                    wv_t = wst.tile([P, NDC, MD], f32r, tag="wv")
                    nc.sync.dma_start(wv_t[:], WV.rearrange("(c p) m -> p c m", p=P))
                for sc in range(SB // P):
                    gsc = sb * (SB // P) + sc
                    ps = pp.tile([P, SB], f32)
                    for dc in range(NDC):
                        nc.tensor.matmul(
                            ps[:],
                            (xt_t[:, dc, sc * P:(sc + 1) * P]),
                            (wv_t[:, dc, :]),
                            start=(dc == 0),
                            stop=False,
                        )
                    nc.tensor.matmul(
                        ps[:], (ones_t[:, :P]), (bvt[:]), start=False, stop=True
                    )
                    nc.scalar.copy(
                        vaug[:, gsc, :, 0:DK],
                        ps.rearrange("p (h d) -> p h d", h=HPC),
                    )
                    nc.gpsimd.memset(vaug[:, gsc, :, DK:DK + 1].bitcast(f32), 1.0)

                # ---------- attention for qsb = sb ----------
                # Head pairs share the 128-partition Q^T/K^T chunks: even head
                # on PE rows 0-63, odd head on rows 64-127 (distinct row
                # groups -> the two score matmuls run concurrently).
                # Causal column skipping: diagonal chunk i only needs
                # sq-cols >= 128*i; keep matmul N >= 256 for fp32r full rate.
                qsb = sb
                nchunks = 4 * qsb + 4
                for hp in range(HPC // 2):
                    po_t = [pop.tile([DK + 1, SB], f32, tag="po", name=f"po{e}")
                            for e in range(2)]
                    first = [True, True]
                    for c in range(nchunks):
                        i = c - 4 * qsb
                        # computed score/exp col range (rounded down to >=256 N)
                        s0 = 0 if i < 1 else min(P * i, 2 * P)
                        sp = psc.tile([P, 2 * SB], f32, tag="sp")
                        for e in range(2):
                            off = e * DK
                            nc.tensor.matmul(
                                sp[:, e * SB + s0:(e + 1) * SB],
                                (kt[off:off + DK, hp, c * P:(c + 1) * P]),
                                (qt_t[off:off + DK, hp, s0:]),
                                start=True,
                                stop=True,
                            )
                        at_g = attnp.tile([P, 2 * SB], f32r)
                        if s0:
                            for e in range(2):
                                nc.scalar.activation(
                                    at_g[:, e * SB + s0:(e + 1) * SB],
                                    sp[:, e * SB + s0:(e + 1) * SB],
                                    AF.Exp, scale=0.125,
                                )
                        else:
                            nc.scalar.activation(at_g[:], sp[:], AF.Exp, scale=0.125)
                        for e in range(2):
                            base = e * SB
                            if i >= 0:
                                if i == 3:
                                    # AV reads [256:512); zero the non-causal
                                    # 256:384 block the N>=256 rule included
                                    nc.gpsimd.memset(
                                        at_g[:, base + 256:base + 384].bitcast(f32),
                                        0.0,
                                    )
                                d0 = base + P * i
                                nc.vector.tensor_mul(
                                    at_g[:, d0:d0 + P],
                                    at_g[:, d0:d0 + P],
                                    masks_t[:],
                                )
                            nc.tensor.matmul(
                                po_t[e][:, s0:],
                                (vaug[:, c, 2 * hp + e, :]),
                                (at_g[:, base + s0:base + SB]),
                                start=first[e],
                                stop=(c == nchunks - 1),
                            )
                            first[e] = False
                    for e in range(2):
                        off = e * DK
                        rs = rsp.tile([1, SB], f32r)
                        nc.vector.reciprocal(rs[:], po_t[e][DK:DK + 1, :])
                        pn = psc.tile([DK, SB], f32, tag="sp")
                        nc.tensor.matmul(
                            pn[:], (ones_t[:, :DK]), (rs[:]), start=True, stop=True
                        )
                        at_sl = at[off:off + DK, hp, qsb * SB:(qsb + 1) * SB]
                        nc.scalar.copy(at_sl, po_t[e][0:DK, :])
                        nc.vector.tensor_mul(at_sl, at_sl, pn[:])

        # ---------- output projection Y^T = WO^T @ A^T ----------
        with (
            tc.tile_pool(name="wo", bufs=1) as wop,
            tc.tile_pool(name="yb", bufs=6) as ybp,
            tc.tile_pool(name="py", bufs=4, space="PSUM") as pyp,
        ):
            wo_t = wop.tile([P, NMC, D], f32r)
            nc.sync.dma_start(wo_t[:], WO.rearrange("(c p) d -> p c d", p=P))
            for sb in range(NSB):
                for dc in range(NDC):
                    ps = pyp.tile([P, SB], f32)
                    for hcc in range(NMC):
                        nc.tensor.matmul(
                            ps[:],
                            (wo_t[:, hcc, dc * P:(dc + 1) * P]),
                            (at[:, hcc, sb * SB:(sb + 1) * SB]),
                            start=(hcc == 0),
                            stop=(hcc == NMC - 1),
                        )
                    yb = ybp.tile([P, SB], f32)
                    nc.vector.tensor_scalar_add(yb[:], ps[:], bot[:, dc:dc + 1])
                    nc.sync.dma_start(
                        YT[dc * P:(dc + 1) * P, sb * SB:(sb + 1) * SB], yb[:]
                    )
    nc.finalize()
    return nc


def _masks():
    p = np.arange(P)[:, None]
    j = np.arange(P)[None, :]
    return (p <= j).astype(np.float32)


def _in_maps(X, Wq, bq, Wk, bk, Wv, bv, Wo, bo):
    masks = _masks()
    zeros_bo = np.zeros_like(bo)
    maps = []
    for core in range(8):
        b, hg = core // 2, core % 2
        sl = slice(hg * MD, (hg + 1) * MD)
        maps.append({
            "XT": np.ascontiguousarray(X[b].T),
            "WQ": np.ascontiguousarray(Wq[:, sl]),
            "WK": np.ascontiguousarray(Wk[:, sl]),
            "WV": np.ascontiguousarray(Wv[:, sl]),
            "WO": np.ascontiguousarray(Wo[sl, :]),
            "BQ": np.ascontiguousarray(bq[sl]),
            "BK": np.ascontiguousarray(bk[sl]),
            "BV": np.ascontiguousarray(bv[sl]),
            "BO": bo if hg == 0 else zeros_bo,
            "MASKS": masks,
        })
    return maps


_LAST_RESULTS = None


def kernel(X, Wq, bq, Wk, bk, Wv, bv, Wo, bo):
    global _LAST_RESULTS
    _ensure_path()
    from concourse import bass_utils

    args = [np.ascontiguousarray(np.asarray(a, dtype=np.float32))
            for a in (X, Wq, bq, Wk, bk, Wv, bv, Wo, bo)]
    if "nc" not in _CACHE:
        _CACHE["nc"] = _build()
    nc = _CACHE["nc"]
    res = bass_utils.run_bass_kernel_spmd(nc, _in_maps(*args), core_ids=list(range(8)))
    _LAST_RESULTS = res
    out = np.empty((B, S, D), dtype=np.float32)
    for b in range(B):
        out[b] = (res.results[2 * b]["YT"] + res.results[2 * b + 1]["YT"]).T
    return out



# revision 31
# speedup vs baseline: 1.1661x; 1.1661x over previous
"""Trainium2 Bass kernel for 16-head causal MHA (B=4, S=2048, D=1024).

Sharding: 8 cores = 4 batches x 2 head-groups (8 heads each).
Per core (batch b, head-group hg):
  inputs:  XT = X[b].T [1024,2048], WQ/WK/WV column shards [1024,512],
           WO row shard [512,1024], bias shards, causal mask tile.
  output:  YT = (O_hg @ WO_hg + bo*[hg==0]).T  [1024, 2048]  (partial)
Host combine: Y[b] = (YT[2b] + YT[2b+1]).T

On-core dataflow:
  Q^T,K^T [512,2048] fp32r and V [2048,512] via fp32r matmuls.
  scores^T[sk,sq] = K_h @ Q_h^T (fp32r, N>=256 diagonal trim); exp on ACT
  (scale=1/8 folded) -> A^T in bf16; causal mask multiplied on diagonal
  128-blocks (DVE, bf16 2x).
  AV in the M=128 form: O[sq,dv+1] += A^T-chunk(stationary) @ [V|1](moving)
  in bf16 (N=65 -> half the PE rows of the V^T@A^T form); col 64 gives the
  softmax denominators for free.
  Normalize on DVE (per-partition reciprocal scale), O -> O^T via PE
  transposes (identity; 4 blocks share one lazily-zeroed PSUM bank),
  Y^T = WO^T @ O^T in bf16.
  Projection / output-projection matmuls are interleaved into the attention
  chunk stream as fillers so the PE never stalls on the ACT exp chain.
"""

import sys
from collections import deque

import numpy as np

_REPO = "/opt/trn_rl_repo"

B, S, D = 4, 2048, 1024
H, DK = 16, 64
HPC = 8            # heads per core
MD = HPC * DK      # 512: per-core head width
P = 128
SB = 512           # s-block
NSB = S // SB      # 4
NDC = D // P       # 8
NMC = MD // P      # 4
NSC = S // P       # 16

_CACHE = {}
_DEBUG = False
_DEBUG_SB = 0


def _ensure_path():
    try:
        import concourse  # noqa: F401
    except ImportError:
        if _REPO not in sys.path:
            sys.path.insert(0, _REPO)


def _build():
    _ensure_path()
    from contextlib import ExitStack

    import concourse.bass as bass  # noqa: F401
    import concourse.mybir as mybir
    import concourse.tile as tile
    from concourse import bacc

    dt = mybir.dt
    f32 = dt.float32
    f32r = dt.float32r
    bf16 = dt.bfloat16
    AF = mybir.ActivationFunctionType

    nc = bacc.Bacc(None, target_bir_lowering=False)
    XT = nc.dram_tensor("XT", [D, S], f32r, kind="ExternalInput")
    WQ = nc.dram_tensor("WQ", [D, MD], f32r, kind="ExternalInput")
    WK = nc.dram_tensor("WK", [D, MD], f32r, kind="ExternalInput")
    WV = nc.dram_tensor("WV", [D, MD], f32r, kind="ExternalInput")
    WO = nc.dram_tensor("WO", [MD, D], f32, kind="ExternalInput")
    BQ = nc.dram_tensor("BQ", [MD], f32, kind="ExternalInput")
    BK = nc.dram_tensor("BK", [MD], f32, kind="ExternalInput")
    BV = nc.dram_tensor("BV", [MD], f32r, kind="ExternalInput")
    BO = nc.dram_tensor("BO", [D], f32, kind="ExternalInput")
    MASKS = nc.dram_tensor("MASKS", [P, P], f32, kind="ExternalInput")
    IDENT = nc.dram_tensor("IDENT", [P, P], f32, kind="ExternalInput")
    YT = nc.dram_tensor("YT", [D, S], bf16, kind="ExternalOutput")
    if _DEBUG:
        DQT = nc.dram_tensor("DQT", [P, NMC, SB], f32r, kind="ExternalOutput")
        DKT = nc.dram_tensor("DKT", [P, NMC, S], f32r, kind="ExternalOutput")
        DVAUG = nc.dram_tensor("DVAUG", [P, NSC, HPC, DK + 1], bf16, kind="ExternalOutput")
        DATG = nc.dram_tensor("DATG", [P, 2, SB], bf16, kind="ExternalOutput")
        DOSB = nc.dram_tensor("DOSB", [P, 4, HPC, DK], bf16, kind="ExternalOutput")
        DOT = nc.dram_tensor("DOT", [P, NMC, SB], bf16, kind="ExternalOutput")

    with ExitStack() as ctx:
        ctx.enter_context(nc.allow_low_precision(reason="fp32r/bf16 pipeline"))
        tc = ctx.enter_context(tile.TileContext(nc))
        consts = ctx.enter_context(tc.tile_pool(name="consts", bufs=1))
        qkv = ctx.enter_context(tc.tile_pool(name="qkv", bufs=1))
        wst = ctx.enter_context(tc.tile_pool(name="wst", bufs=1))
        xtp = ctx.enter_context(tc.tile_pool(name="xt", bufs=2))
        wotp = ctx.enter_context(tc.tile_pool(name="wotmp", bufs=1))
        qtp = ctx.enter_context(tc.tile_pool(name="qt", bufs=2))
        attnp = ctx.enter_context(tc.tile_pool(name="attn", bufs=3))
        osbp = ctx.enter_context(tc.tile_pool(name="osb", bufs=2))
        otp = ctx.enter_context(tc.tile_pool(name="ot", bufs=4))
        recp = ctx.enter_context(tc.tile_pool(name="rec", bufs=2))
        ybp = ctx.enter_context(tc.tile_pool(name="yb", bufs=2))
        pps = ctx.enter_context(tc.tile_pool(name="pps", bufs=2, space="PSUM"))
        psp = ctx.enter_context(tc.tile_pool(name="psp", bufs=2, space="PSUM"))
        pav = ctx.enter_context(tc.tile_pool(name="pav", bufs=2, space="PSUM"))

        # Dummy first ACT op: walrus attaches the ACT table-load pseudo to the
        # first activation; keep its sync-wait list minimal.
        dummy = consts.tile([1, 16], f32)
        nc.vector.memset(dummy[:], 0.0)
        nc.scalar.activation(dummy[:], dummy[:], AF.Exp)
        nc.scalar.activation(dummy[:], dummy[:], AF.Identity)

        ones_t = consts.tile([1, P], f32r)
        nc.vector.memset(ones_t[:].bitcast(f32), 1.0)

        kt = qkv.tile([P, NMC, S], f32r)              # K^T  (m-chunk, sk)
        vaug = qkv.tile([P, NSC, HPC, DK + 1], bf16)  # V per s-chunk + ones col
        wo_bf = qkv.tile([P, NMC, D], bf16)

        # ---------- setup DMAs, ordered so the PE starts early --------------
        xt_t = [None] * NSB
        xt_t[0] = xtp.tile([P, NDC, SB], f32r, name="xtblk")
        wq_t = wst.tile([P, NDC, MD], f32r, tag="wq")
        wk_t = wst.tile([P, NDC, MD], f32r, tag="wk")
        wv_t = wst.tile([P, NDC, MD], f32r, tag="wv")
        xt_r = XT.rearrange("(c p) s -> p c s", p=P)
        wq_r = WQ.rearrange("(c p) m -> p c m", p=P)
        wk_r = WK.rearrange("(c p) m -> p c m", p=P)
        wv_r = WV.rearrange("(c p) m -> p c m", p=P)
        # xt chunks and per-mc W strips interleaved: Q(mc)/K(mc) chains can
        # start as soon as xt + strip mc land; wv right after so the V
        # projection (phase-0 filler) is ready before the first AV needs it.
        def w_strip(w_t, w_r, mc):
            nc.sync.dma_start(
                w_t[:, :, mc * P:(mc + 1) * P], w_r[:, :, mc * P:(mc + 1) * P]
            )

        # xt0 + first Q/K strips interleaved, then constants, then the rest;
        # wv early so the V projection (phase-0 filler) is ready in time.
        for dc in range(NDC):
            nc.sync.dma_start(xt_t[0][:, dc, :], xt_r[:, dc, 0:SB])
            if dc == 0:
                w_strip(wq_t, wq_r, 0)
            if dc == 1:
                w_strip(wk_t, wk_r, 0)
        bqt = consts.tile([P, NMC], f32)
        nc.sync.dma_start(bqt[:], BQ.rearrange("(c p) -> p c", p=P))
        bkt = consts.tile([P, NMC], f32)
        nc.sync.dma_start(bkt[:], BK.rearrange("(c p) -> p c", p=P))
        masks_f = consts.tile([P, P], f32)
        nc.sync.dma_start(masks_f[:], MASKS[:, :])
        bvt = consts.tile([1, MD], f32r)
        nc.sync.dma_start(bvt[:], BV[None, :])
        w_strip(wq_t, wq_r, 1)
        w_strip(wk_t, wk_r, 1)
        nc.sync.dma_start(wv_t[:], wv_r[:, :, :])
        for mc in range(2, NMC):
            w_strip(wq_t, wq_r, mc)
            w_strip(wk_t, wk_r, mc)
        xt_t[1] = xtp.tile([P, NDC, SB], f32r, name="xtblk")
        nc.sync.dma_start(xt_t[1][:], xt_r[:, :, SB:2 * SB])

        bot = consts.tile([P, NDC], f32)
        nc.sync.dma_start(bot[:], BO.rearrange("(c p) -> p c", p=P))
        # WO: load fp32 into a scratch slot, convert to bf16 (needed late).
        wo_tmp = wotp.tile([P, NMC, D], f32, name="wotmp")
        nc.sync.dma_start(wo_tmp[:], WO.rearrange("(c p) d -> p c d", p=P))
        ident_f = consts.tile([P, P], f32)
        nc.sync.dma_start(ident_f[:], IDENT[:, :])
        masks_b = consts.tile([P, P], bf16)
        nc.scalar.copy(masks_b[:], masks_f[:])
        ident_b = consts.tile([P, P], bf16)
        nc.scalar.copy(ident_b[:], ident_f[:])
        for hc in range(NMC):
            nc.scalar.copy(wo_bf[:, hc, :], wo_tmp[:, hc, :])

        # ---------- emitters ------------------------------------------------
        # filler units: (estimated_pe_ns, closure)
        U = 0.41667  # ns per PE row at full clock

        def qk_units(sb, qt):
            """Q/K projection for s-block sb: 16 (est, closure) units."""
            xt = xt_t[sb]

            def qk_half(w_t, bias_t, out_t, mc, half, ps_box):
                def run():
                    if half == 0:
                        ps_box[0] = pps.tile([P, SB], f32, name="ps")
                    ps = ps_box[0]
                    for dc in range(4 * half, 4 * half + 4):
                        nc.tensor.matmul(
                            ps[:],
                            (w_t[:, dc, mc * P:(mc + 1) * P]),
                            (xt[:, dc, :]),
                            start=(dc == 0),
                            stop=(dc == NDC - 1),
                        )
                    if half == 1:
                        nc.vector.tensor_scalar_add(
                            out_t[:, mc, :] if out_t is not kt
                            else kt[:, mc, sb * SB:(sb + 1) * SB],
                            ps[:], bias_t[:, mc:mc + 1],
                        )
                return run

            out = []
            for mc in range(NMC):
                box_q, box_k = [None], [None]
                out.append((4 * SB * U, None, qk_half(wq_t, bqt, qt, mc, 0, box_q)))
                out.append((4 * SB * U, None, qk_half(wq_t, bqt, qt, mc, 1, box_q)))
                out.append((4 * SB * U, None, qk_half(wk_t, bkt, kt, mc, 0, box_k)))
                out.append((4 * SB * U, None, qk_half(wk_t, bkt, kt, mc, 1, box_k)))
            return out

        def v_units(sb):
            """V projection for s-block sb: 8 (est, closure) units."""
            xt = xt_t[sb]

            def v_half(sc, half, ps_box):
                gsc = sb * (SB // P) + sc

                def run():
                    if half == 0:
                        ps_box[0] = pps.tile([P, SB], f32, name="ps")
                    ps = ps_box[0]
                    for dc in range(4 * half, 4 * half + 4):
                        nc.tensor.matmul(
                            ps[:],
                            (xt[:, dc, sc * P:(sc + 1) * P]),
                            (wv_t[:, dc, :]),
                            start=(dc == 0),
                            stop=False,
                        )
                    if half == 1:
                        nc.tensor.matmul(
                            ps[:], (ones_t[:, :P]), (bvt[:]),
                            start=False, stop=True,
                        )
                        nc.vector.tensor_scalar_add(
                            vaug[:, gsc, :, 0:DK],
                            ps.rearrange("p (h d) -> p h d", h=HPC),
                            0.0,
                        )
                        nc.gpsimd.memset(vaug[:, gsc, :, DK:DK + 1], 1.0)
                return run

            out = []
            for sc in range(SB // P):
                box_v = [None]
                # deadline: B(hp0, c=4*sb+sc) consumes vaug chunk 4*sb+sc
                dl = 4 * sb + sc
                out.append((4 * SB * U, dl, v_half(sc, 0, box_v)))
                out.append((5 * SB * U, dl, v_half(sc, 1, box_v)))
            return out

        def wo_units(sb, ot):
            """Output projection for s-block sb: 8 (est, closure) units."""
            out = []

            def one(dc):
                def run():
                    ps = pps.tile([P, SB], f32, name="ps")
                    for hc in range(NMC):
                        nc.tensor.matmul(
                            ps[:],
                            (wo_bf[:, hc, dc * P:(dc + 1) * P]),
                            (ot[:, hc, :]),
                            start=(hc == 0),
                            stop=(hc == NMC - 1),
                        )
                    yb = ybp.tile([P, SB], bf16, name="yb")
                    nc.vector.tensor_scalar_add(yb[:], ps[:], bot[:, dc:dc + 1])
                    nc.sync.dma_start(
                        YT[dc * P:(dc + 1) * P, sb * SB:(sb + 1) * SB], yb[:]
                    )
                return run

            for dc in range(NDC):
                out.append((4 * SB * U, None, one(dc)))
            return out

        # ---------- main phases --------------------------------------------
        deferred_wo = []          # (sb, ot) pairs whose WO is deferred to p3
        ACT_C = 0.8333            # ns per ACT element

        # Q/K proj(0) runs standalone (nothing else for the PE yet); V(0)
        # goes to the phase-0 fillers (wv lands after the Q/K strips).
        qt_cur = qtp.tile([P, NMC, SB], f32r, name="qt")
        for _, _, u in qk_units(0, qt_cur):
            u()

        for sb in range(NSB):
            qsb = sb
            # stream XT for sb+2 (xt pool bufs=2; sb,sb+1 already resident)
            if sb + 2 < NSB:
                xt_t[sb + 2] = xtp.tile([P, NDC, SB], f32r, name="xtblk")
                nc.sync.dma_start(xt_t[sb + 2][:], xt_r[:, :, (sb + 2) * SB:(sb + 3) * SB])

            # filler inventory for this phase: this block's V projection
            # (deadline-paced, just in time for the diagonal AVs), the next
            # block's Q/K projection, and in the last phase all deferred WO.
            fillers = deque()
            fillers.extend(v_units(sb))
            qt_next = None
            if sb + 1 < NSB:
                qt_next = qtp.tile([P, NMC, SB], f32r, name="qt")
                fillers.extend(qk_units(sb + 1, qt_next))
            if sb == NSB - 1:
                for dsb, dot in deferred_wo:
                    fillers.extend(wo_units(dsb, dot))

            qt = qt_cur
            osb = osbp.tile([P, 4, HPC, DK], bf16, name="osb")
            ot = otp.tile([P, NMC, SB], bf16, name="ot")
            if _DEBUG and sb == _DEBUG_SB:
                nc.sync.dma_start(DQT[:, :, :], qt[:])

            nchunks = 4 * qsb + 4
            # deficit-paced filling: per chunk, ACT exp cost minus the PE
            # work of the chunk itself; scaled so the filler supply lasts
            # exactly to the end of the phase.
            def chunk_deficit(c):
                i = c - 4 * qsb
                s0 = 0 if i < 1 else min(P * i, 2 * P)
                n_av = 2 * (4 - max(i, 0))
                act = 2 * (SB - s0) * ACT_C + 185.0
                pe = (2 * (SB - s0) + n_av * 65) * U
                return max(act - pe, 0.0)

            tot_deficit = sum(chunk_deficit(c) for c in range(nchunks)) * NMC
            tot_fill = sum(est for est, _, _ in fillers)
            dscale = min(1.0, tot_fill / max(tot_deficit, 1.0))
            budget = [0.0]

            def pop_filler():
                est, _, u = fillers.popleft()
                u()
                budget[0] -= est

            def fill(d, c_slot):
                budget[0] += d * dscale
                while fillers and fillers[0][1] is not None and fillers[0][1] <= c_slot:
                    pop_filler()
                while fillers and budget[0] >= fillers[0][0] * 0.5:
                    pop_filler()

            def emit_a(hp, c):
                i = c - 4 * qsb
                s0 = 0 if i < 1 else min(P * i, 2 * P)
                sp = psp.tile([P, 2, SB], f32, tag="sp")
                for e in range(2):
                    off = e * DK
                    nc.tensor.matmul(
                        sp[:, e, s0:],
                        (kt[off:off + DK, hp, c * P:(c + 1) * P]),
                        (qt[off:off + DK, hp, s0:]),
                        start=True,
                        stop=True,
                    )
                at_g = attnp.tile([P, 2, SB], bf16)
                nc.scalar.activation(
                    at_g[:, :, s0:], sp[:, :, s0:], AF.Exp, scale=0.125
                )
                if i >= 0:
                    d0 = P * i
                    for e in range(2):
                        nc.vector.tensor_mul(
                            at_g[:, e, d0:d0 + P],
                            at_g[:, e, d0:d0 + P],
                            masks_b[:],
                        )
                if _DEBUG and qsb == _DEBUG_SB and hp == 0 and c == 0:
                    nc.sync.dma_start(DATG[:, :, :], at_g[:])
                return at_g

            o_map = {}

            def make_b(hp, c, at_g):
                i = c - 4 * qsb

                def run():
                    if c == 0:
                        o_map[hp] = [
                            pav.tile([P, 4, DK + 1], f32, name="oacc")
                            for _ in range(2)
                        ]
                    o_e = o_map[hp]
                    # One PSUM accumulation group per (hp, e) bank: start=1
                    # lazily zeroes the whole 2KB zero region, so only the
                    # very first matmul starts and only the last one stops.
                    # (Interleaved per-jj groups in one bank corrupt on HW.)
                    for e in range(2):
                        for jj in range(max(i, 0), 4):
                            nc.tensor.matmul(
                                o_e[e][:, jj, :],
                                (at_g[:, e, jj * P:(jj + 1) * P]),
                                (vaug[:, c, 2 * hp + e, :]),
                                start=(c == 0 and jj == 0),
                                stop=(c == 4 * qsb + 3),
                            )
                return run

            def epilogue(hp):
                # reciprocal + normalize + transpose for one head pair
                o_e = o_map.pop(hp)
                rec = recp.tile([P, 2, 4], f32)
                for e in range(2):
                    nc.vector.reciprocal(rec[:, e, :], o_e[e][:, :, DK])
                for e in range(2):
                    for jj in range(4):
                        nc.vector.tensor_scalar_mul(
                            osb[:, jj, 2 * hp + e, :],
                            o_e[e][:, jj, 0:DK],
                            rec[:, e, jj:jj + 1],
                        )
                # PE transpose of the 4 [128,128] blocks into one PSUM bank
                # (single lazy-zeroed group), then one DVE copy to ot.
                tp = pps.tile([P, SB], f32, name="ps")
                tpb = tp[:].bitcast(bf16)
                for jj in range(4):
                    nc.tensor.matmul(
                        tpb[:, jj * P:(jj + 1) * P],
                        osb[:, jj, 2 * hp:2 * hp + 2, :],
                        ident_b[:],
                        is_transpose=True,
                        start=(jj == 0),
                        stop=(jj == 3),
                    )
                nc.vector.tensor_scalar_add(ot[:, hp, :], tpb[:, 0:SB], 0.0)

            # cross-hp pipelined chunk stream: B(k) is emitted after A(k+1),
            # and each hp's epilogue right after its last B, which already
            # overlaps the next hp's first scores.
            prev = None
            for hp in range(NMC):
                for c in range(nchunks):
                    at_g = emit_a(hp, c)
                    fill(chunk_deficit(c), c if hp == 0 else nchunks)
                    if prev is not None:
                        ph, pc, pb = prev
                        pb()
                        if pc == nchunks - 1:
                            epilogue(ph)
                    prev = (hp, c, make_b(hp, c, at_g))
            fill(500.0, nchunks)
            ph, pc, pb = prev
            pb()
            epilogue(ph)

            # drain leftover fillers
            while fillers:
                pop_filler()

            if _DEBUG and sb == _DEBUG_SB:
                nc.sync.dma_start(DOSB[:, :, :, :], osb[:])
                nc.sync.dma_start(DOT[:, :, :], ot[:])
            if _DEBUG and sb == NSB - 1:
                nc.sync.dma_start(DKT[:, :, :], kt[:])
                nc.sync.dma_start(DVAUG[:, :, :, :], vaug[:])
            deferred_wo.append((sb, ot))
            qt_cur = qt_next

        # tail: WO for the last s-block
        sb3, ot3 = deferred_wo[-1]
        for _, _, u in wo_units(sb3, ot3):
            u()
        # earlier blocks' WO ran as fillers in phase 3
    nc.finalize()
    return nc


def _masks():
    p = np.arange(P)[:, None]
    j = np.arange(P)[None, :]
    return (p <= j).astype(np.float32)


def _in_maps(X, Wq, bq, Wk, bk, Wv, bv, Wo, bo):
    masks = _masks()
    zeros_bo = np.zeros_like(bo)
    maps = []
    for core in range(8):
        b, hg = core // 2, core % 2
        sl = slice(hg * MD, (hg + 1) * MD)
        maps.append({
            "XT": np.ascontiguousarray(X[b].T),
            "WQ": np.ascontiguousarray(Wq[:, sl]),
            "WK": np.ascontiguousarray(Wk[:, sl]),
            "WV": np.ascontiguousarray(Wv[:, sl]),
            "WO": np.ascontiguousarray(Wo[sl, :]),
            "BQ": np.ascontiguousarray(bq[sl]),
            "BK": np.ascontiguousarray(bk[sl]),
            "BV": np.ascontiguousarray(bv[sl]),
            "BO": bo if hg == 0 else zeros_bo,
            "MASKS": masks,
            "IDENT": np.eye(P, dtype=np.float32),
        })
    return maps


_LAST_RESULTS = None


def kernel(X, Wq, bq, Wk, bk, Wv, bv, Wo, bo):
    global _LAST_RESULTS
    _ensure_path()
    from concourse import bass_utils

    args = [np.ascontiguousarray(np.asarray(a, dtype=np.float32))
            for a in (X, Wq, bq, Wk, bk, Wv, bv, Wo, bo)]
    if "nc" not in _CACHE:
        _CACHE["nc"] = _build()
    nc = _CACHE["nc"]
    res = bass_utils.run_bass_kernel_spmd(nc, _in_maps(*args), core_ids=list(range(8)))
    _LAST_RESULTS = res
    out = np.empty((B, S, D), dtype=np.float32)
    for b in range(B):
        out[b] = (res.results[2 * b]["YT"] + res.results[2 * b + 1]["YT"]).T
    return out
